# revision 18
# baseline (speedup 1.0000x reference)
"""AlignmentContrastiveLoss on 8 Trainium2 NeuronCores.

Math notes (derived from the reference):
  - participating nodes are exactly those with >=1 positive partner, and every
    participating node is conserved (pos_full requires cons_i & cons_j). Hence
    within participating x participating, valid = (pos|neg)&part&~diag reduces
    to just ~same_graph (diag is same_graph with itself).
  - logits never exceed 1/T = 10, so sum-exp needs no max subtraction.
  - the device only computes U_i = sum_j exp(10*(E_i.E_j - 3*[g_i==g_j])) over
    the gathered participating set; the -30 logit penalty (exp ~ 1e-9 relative)
    implements the mask, kills the diagonal, and kills padded columns (whose
    one-hot section is all-ones). Everything else - the positive-pair (1-S)
    term, per-row positive counts, log, and the final scalar - is O(N + pairs)
    host work.

Sharding: rows of the gathered similarity matrix are split evenly over the 8
cores; each core computes full row-sums for its slice, so no collective is
needed - the host concatenates the 8 partial U vectors.
"""

from contextlib import ExitStack

import ml_dtypes
import numpy as np

import concourse.bass as bass
import concourse.mybir as mybir
import concourse.tile as tile
from concourse import bacc
from concourse.bass_utils import run_bass_kernel_spmd

N_CORES = 8
TEMP = 0.1
EPS = 1e-12
PEN = 3.0  # pre-scale graph penalty; exp applies scale=1/T -> -30 in logit space
DEXP = 16  # one-hot graph dims (graph_ids in [0, 16))
NTILE = 512  # matmul free-dim tile
CHUNK = 4  # n-tiles per PSUM buffer / ACT call
USE_FP8 = True  # fp8e4m3 DoubleRow for the 256-dim emb contraction

_programs: dict[tuple, bass.Bass] = {}


def _build_program(npad: int, repeat: int = 1, fp8: bool = USE_FP8) -> bass.Bass:
    """One SPMD Bass program: each core gets the full column matrix plus its
    own row slice, and writes U partial row-sums [R, 1].

    fp8=True packs the 256-dim emb contraction as fp8e4 DoubleRow (one matmul
    per n-tile instead of two); the graph-penalty K=16 matmul stays bf16.
    repeat > 1 wraps the compute body in a hardware loop (benchmarking only)."""
    rows = npad // N_CORES
    bf = mybir.dt.bfloat16
    f8 = mybir.dt.float8e4
    f32 = mybir.dt.float32
    Exp = mybir.ActivationFunctionType.Exp

    nc = bacc.Bacc(
        "TRN2", target_bir_lowering=False, debug=False, num_devices=N_CORES
    )
    if fp8:
        yt8 = nc.declare_dram_parameter("yt8", [128, 2, npad], f8, isOutput=False)
        ytg = nc.declare_dram_parameter("ytg", [DEXP, npad], bf, isOutput=False)
        xs8 = nc.declare_dram_parameter("xs8", [128, 2, rows], f8, isOutput=False)
        xsg = nc.declare_dram_parameter("xsg", [DEXP, rows], bf, isOutput=False)
    else:
        yt = nc.declare_dram_parameter("yt", [256 + DEXP, npad], bf, isOutput=False)
        xs = nc.declare_dram_parameter("xs", [256 + DEXP, rows], bf, isOutput=False)
    u = nc.declare_dram_parameter("u", [rows, 1], f32, isOutput=True)

    n_tiles = npad // NTILE
    chunks = [
        (c0 * NTILE, min(CHUNK, n_tiles - c0) * NTILE)
        for c0 in range(0, n_tiles, CHUNK)
    ]
    m_tiles = [(m0, min(128, rows - m0)) for m0 in range(0, rows, 128)]

    with tile.TileContext(nc) as tc, ExitStack() as ctx:
        const = ctx.enter_context(tc.tile_pool(name="const", bufs=1))
        psum = ctx.enter_context(
            tc.tile_pool(name="psum", bufs=2, space=bass.MemorySpace.PSUM)
        )
        scratch = ctx.enter_context(tc.tile_pool(name="scratch", bufs=2))
        accp = ctx.enter_context(tc.tile_pool(name="acc", bufs=2))

        # Warm the exp table while DMAs run.
        dummy_in = const.tile([128, 8], f32)
        nc.vector.memset(dummy_in[:], 0.0)
        dummy_out = const.tile([128, 8], bf)
        nc.scalar.activation(dummy_out[:], dummy_in[:], Exp)

        # Row-slice operands (lhsT). Split loads across the two HWDGE rings
        # (SP=nc.sync, ACT=nc.scalar) so the head isn't serialized on one ring.
        if fp8:
            x8 = const.tile([128, 2, rows], f8)
            nc.sync.dma_start(x8[:], xs8[:, :, :])
            xg = const.tile([DEXP, rows], bf)
            nc.scalar.dma_start(xg[:], xsg[:, :])
            xemb = (x8,)
        else:
            x0 = const.tile([128, rows], bf)
            nc.sync.dma_start(x0[:], xs[0:128, :])
            x1 = const.tile([128, rows], bf)
            nc.scalar.dma_start(x1[:], xs[128:256, :])
            xg = const.tile([DEXP, rows], bf)
            nc.scalar.dma_start(xg[:], xs[256 : 256 + DEXP, :])
            xemb = (x0, x1)

        # Column operands (rhs), one DMA per (k-tile, chunk); alternate rings
        # by chunk so both rings stream concurrently.
        ycols = []
        for ci, (c0, cw) in enumerate(chunks):
            ring = nc.sync if ci % 2 == 0 else nc.scalar
            if fp8:
                y8 = const.tile([128, 2, cw], f8, tag=f"y8_{c0}")
                ring.dma_start(y8[:], yt8[:, :, c0 : c0 + cw])
                yg = const.tile([DEXP, cw], bf, tag=f"yg_{c0}")
                ring.dma_start(yg[:], ytg[:, c0 : c0 + cw])
                ycols.append((y8, yg))
            else:
                y0 = const.tile([128, cw], bf, tag=f"y0_{c0}")
                ring.dma_start(y0[:], yt[0:128, c0 : c0 + cw])
                y1 = const.tile([128, cw], bf, tag=f"y1_{c0}")
                ring.dma_start(y1[:], yt[128:256, c0 : c0 + cw])
                y2 = const.tile([DEXP, cw], bf, tag=f"y2_{c0}")
                ring.dma_start(y2[:], yt[256 : 256 + DEXP, c0 : c0 + cw])
                ycols.append((y0, y1, y2))

        def body():
            _emit_compute(
                nc, tc, m_tiles, chunks, ycols, xemb, xg, u,
                psum, scratch, accp, fp8,
            )

        if repeat == 1:
            body()
        else:
            with tc.For_i(0, repeat, 1):
                body()

    nc.compile()
    return nc


def _emit_compute(nc, tc, m_tiles, chunks, ycols, xemb, xg, u, psum, scratch, accp, fp8):
    bf = mybir.dt.bfloat16
    f32 = mybir.dt.float32
    Exp = mybir.ActivationFunctionType.Exp
    for m0, mw in m_tiles:
        acc = accp.tile([128, len(chunks)], f32)
        for ci, (c0, cw) in enumerate(chunks):
            ps = psum.tile([128, CHUNK * NTILE], f32, tag="ps")
            for t in range(cw // NTILE):
                nsl = slice(t * NTILE, (t + 1) * NTILE)
                if fp8:
                    (x8,) = xemb
                    y8, yg = ycols[ci]
                    nc.tensor.matmul(
                        ps[:mw, nsl],
                        x8[:, :, m0 : m0 + mw],
                        y8[:, :, t * NTILE : (t + 1) * NTILE],
                        start=True, stop=False,
                        perf_mode=mybir.MatmulPerfMode.DoubleRow,
                    )
                else:
                    x0, x1 = xemb
                    y0, y1, yg = ycols[ci]
                    nc.tensor.matmul(
                        ps[:mw, nsl], x0[:, m0 : m0 + mw], y0[:, nsl],
                        start=True, stop=False,
                    )
                    nc.tensor.matmul(
                        ps[:mw, nsl], x1[:, m0 : m0 + mw], y1[:, nsl],
                        start=False, stop=False,
                    )
                nc.tensor.matmul(
                    ps[:mw, nsl], xg[:, m0 : m0 + mw], yg[:, nsl],
                    start=False, stop=True,
                )
            sc = scratch.tile([128, CHUNK * NTILE], bf, tag="sc")
            nc.scalar.activation(
                sc[:mw, :cw], ps[:mw, :cw], Exp,
                scale=1.0 / TEMP,
                accum_out=acc[:mw, ci : ci + 1],
            )
        ured = accp.tile([128, 1], f32, tag="ured")
        nc.vector.tensor_reduce(
            ured[:mw, :], acc[:mw, : len(chunks)],
            axis=mybir.AxisListType.X, op=mybir.AluOpType.add,
        )
        nc.sync.dma_start(u[m0 : m0 + mw, :], ured[:mw, :])


def kernel(embeddings, labels, graph_ids, categories):
    import os
    import time

    _dbg = bool(os.environ.get("KERNEL_DEBUG_TIMING"))
    _t0 = time.time()

    def _mark(msg):
        if _dbg:
            print(f"[kernel] {msg}: {time.time() - _t0:.2f}s", flush=True)

    emb = np.asarray(embeddings, dtype=np.float32)
    lab = np.asarray(labels).astype(np.int64)
    gid = np.asarray(graph_ids).astype(np.int64)
    cat = np.asarray(categories).astype(np.int64)
    n, d = emb.shape
    assert d == 256

    norms = np.linalg.norm(emb, axis=1, keepdims=True)
    e = emb / np.maximum(norms, EPS)

    cons = cat < 3

    # Label groups via sort; a conserved node participates iff its label group
    # has conserved members spanning >=2 distinct graphs.
    order = np.argsort(lab, kind="stable")
    lab_s = lab[order]
    starts = np.flatnonzero(np.r_[True, lab_s[1:] != lab_s[:-1]])
    ends = np.r_[starts[1:], n]

    part_mask = np.zeros(n, dtype=bool)
    cnt = np.zeros(n, dtype=np.int64)  # positive partners per node
    pair_i, pair_j = [], []  # unordered positive pairs
    for s, t in zip(starts, ends):
        idx = order[s:t]
        ci = idx[cons[idx]]
        if len(ci) < 2:
            continue
        gg = gid[ci]
        if (gg == gg[0]).all():
            continue
        part_mask[ci] = True
        # partners: same label, conserved, different graph
        gcounts = {}
        for g in gg:
            gcounts[g] = gcounts.get(g, 0) + 1
        cnt[ci] = len(ci) - np.array([gcounts[g] for g in gg])
        ii, jj = np.triu_indices(len(ci), k=1)
        diff = gg[ii] != gg[jj]
        pair_i.append(ci[ii[diff]])
        pair_j.append(ci[jj[diff]])

    if not pair_i:
        return np.float32(0.0)
    pair_i = np.concatenate(pair_i)
    pair_j = np.concatenate(pair_j)
    n_pairs = len(pair_i)
    if n_pairs == 0:
        return np.float32(0.0)

    _mark("host group prep")
    # Host pair similarities (fp32 like the reference).
    s_pairs = np.einsum("ij,ij->i", e[pair_i], e[pair_j], dtype=np.float64)
    pos_loss = np.sum(1.0 - s_pairs) / n_pairs

    part = np.flatnonzero(part_mask)
    npp = len(part)
    npad = max(NTILE, -(-npp // NTILE) * NTILE)

    # Graph one-hot [16, npad]; padded columns get all-ones so every row
    # sees the -PEN penalty (kills diag, same-graph, and pad columns).
    g_onehot = np.zeros((DEXP, npad), dtype=ml_dtypes.bfloat16)
    g_onehot[gid[part], np.arange(npp)] = 1.0
    g_onehot[:, npp:] = 1.0

    rows = npad // N_CORES
    if USE_FP8:
        f8np = mybir.dt.np(mybir.dt.float8e4)
        e8 = e[part].astype(f8np)  # [npp, 256]
        # DoubleRow packing: [ki, ko, n] = E[n, ki + 128*ko]
        yt8 = np.zeros((128, 2, npad), dtype=f8np)
        yt8[:, :, :npp] = e8.T.reshape(2, 128, npp).transpose(1, 0, 2)
        xg_full = (g_onehot.astype(np.float32) * -PEN).astype(ml_dtypes.bfloat16)
        in_maps = [
            {
                "yt8": yt8,
                "ytg": g_onehot,
                "xs8": np.ascontiguousarray(yt8[:, :, c * rows : (c + 1) * rows]),
                "xsg": np.ascontiguousarray(xg_full[:, c * rows : (c + 1) * rows]),
            }
            for c in range(N_CORES)
        ]
    else:
        ebf = e[part].astype(ml_dtypes.bfloat16)
        yt = np.zeros((256 + DEXP, npad), dtype=ml_dtypes.bfloat16)
        yt[:256, :npp] = ebf.T
        yt[256:, :] = g_onehot
        xt = yt.copy()
        xt[256:, :] = g_onehot.astype(np.float32) * -PEN
        in_maps = [
            {"yt": yt, "xs": np.ascontiguousarray(xt[:, c * rows : (c + 1) * rows])}
            for c in range(N_CORES)
        ]

    _mark("host arrays built")
    key = (npad, USE_FP8)
    nc = _programs.get(key)
    if nc is None:
        nc = _build_program(npad)
        _programs[key] = nc
    _mark("program built")
    res = run_bass_kernel_spmd(nc, in_maps, core_ids=list(range(N_CORES)))
    _mark("device run done")
    u_full = np.concatenate([r["u"].reshape(-1) for r in res.results])[:npp]

    lse = np.log(np.maximum(u_full.astype(np.float64), 1e-300))
    # nce = (sum_i cnt_i * lse_i - sum_ordered_pos logits) / n_pos
    n_pos = 2 * n_pairs
    nce = (np.sum(cnt[part] * lse) - 2.0 * np.sum(s_pairs / TEMP)) / n_pos

    return np.float32(pos_loss + nce)


# revision 38
# speedup vs baseline: 2.2180x; 2.2180x over previous
"""AlignmentContrastiveLoss on 8 Trainium2 NeuronCores.

Math notes (derived from the reference):
  - participating nodes are exactly those with >=1 positive partner, and every
    participating node is conserved (pos_full requires cons_i & cons_j). Hence
    within participating x participating, valid = (pos|neg)&part&~diag reduces
    to just ~same_graph (diag is same_graph with itself).
  - logits never exceed 1/T = 10, so sum-exp needs no max subtraction.
  - the device only computes U_i = sum_j exp(10*(E_i.E_j - 3*[g_i==g_j])) over
    the gathered participating set; the -30 logit penalty (exp ~ 1e-9 relative)
    implements the mask, kills the diagonal, and kills padded columns (whose
    one-hot section is all-ones). Everything else - the positive-pair (1-S)
    term, per-row positive counts, log, and the final scalar - is O(N + pairs)
    host work.

Sharding: rows of the gathered similarity matrix are split evenly over the 8
cores; each core computes full row-sums for its slice, so no collective is
needed - the host concatenates the 8 partial U vectors.
"""

from contextlib import ExitStack

import ml_dtypes
import numpy as np

import concourse.bass as bass
import concourse.mybir as mybir
import concourse.tile as tile
from concourse import bacc
from concourse.bass_utils import run_bass_kernel_spmd

N_CORES = 8
TEMP = 0.1
EPS = 1e-12
PEN = 3.0  # pre-scale graph penalty; exp applies scale=1/T -> -30 in logit space
DEXP = 16  # one-hot graph dims (graph_ids in [0, 16))
NTILE = 512  # matmul free-dim tile
CHUNK = 4  # n-tiles per PSUM buffer / ACT call
USE_FP8 = True  # fp8e4m3 DoubleRow for the 256-dim emb contraction
USE_TRI = True  # triangle scheme: each unordered pair computed once

_programs: dict[tuple, bass.Bass] = {}


def _tri_pairs(npad: int):
    """Work list: upper-triangle (m-tile, n-tile) slots at 512x512-square
    granularity, paired per m-tile for 1024-wide ACT calls, padded so every
    core gets the same number of pairs.

    Slot (mi, ni) covers rows [128mi,128mi+128) x cols [512ni,512ni+512).
    ni == mi//4 is a diagonal-square slot (row-sums only, host skips its
    col-sums); ni > mi//4 is strictly upper (row-sums + col-sums)."""
    m_t = npad // 128
    n_t = npad // NTILE
    pairs = []  # (mi, [ni,...]) with 1 or 2 nis
    for mi in range(m_t):
        nis = list(range(mi // 4, n_t))
        for a in range(0, len(nis), 2):
            pairs.append((mi, nis[a : a + 2]))
    while len(pairs) % N_CORES:
        pairs.append((None, []))
    return pairs


def _build_program_tri(npad: int, repeat: int = 1) -> bass.Bass:
    """Triangle-scheme SPMD program: ppc uniform pair-slots per core, operand
    slabs supplied as data. Outputs per core: ur [128, ppc] row-sums and
    uc [ppc, 1024] column-sums."""
    ppc = len(_tri_pairs(npad)) // N_CORES
    bf = mybir.dt.bfloat16
    f8 = mybir.dt.float8e4
    f32 = mybir.dt.float32
    Exp = mybir.ActivationFunctionType.Exp

    nc = bacc.Bacc(
        "TRN2",
        target_bir_lowering=False,
        debug=False,
        num_devices=N_CORES,
        disable_frame_to_traceback=True,
    )
    xs8p = nc.declare_dram_parameter("xs8p", [128, 2, ppc * 128], f8, isOutput=False)
    xgp = nc.declare_dram_parameter("xgp", [DEXP, ppc * 128], bf, isOutput=False)
    ys8p = nc.declare_dram_parameter("ys8p", [128, 2, ppc * 1024], f8, isOutput=False)
    ygp = nc.declare_dram_parameter("ygp", [DEXP, ppc * 1024], bf, isOutput=False)
    ur = nc.declare_dram_parameter("ur", [128, ppc], f32, isOutput=True)
    # col-sums: one row per slot (= pair half), 4 slots per psum bank group
    n_groups = -(-2 * ppc // 4)
    uc = nc.declare_dram_parameter("uc", [n_groups, 4, NTILE], f32, isOutput=True)

    with tile.TileContext(nc) as tc, ExitStack() as ctx:
        const = ctx.enter_context(tc.tile_pool(name="const", bufs=1))
        psum = ctx.enter_context(
            tc.tile_pool(name="psum", bufs=2, space=bass.MemorySpace.PSUM)
        )
        psumc = ctx.enter_context(
            tc.tile_pool(name="psumc", bufs=2, space=bass.MemorySpace.PSUM)
        )
        scratch = ctx.enter_context(tc.tile_pool(name="scratch", bufs=2))
        accp = ctx.enter_context(tc.tile_pool(name="acc", bufs=2))

        # Warm the exp table while DMAs run.
        dummy_in = const.tile([128, 8], f32)
        nc.vector.memset(dummy_in[:], 0.0)
        dummy_out = const.tile([128, 8], bf)
        nc.scalar.activation(dummy_out[:], dummy_in[:], Exp)

        # 32 ones-columns: each col-sum matmul writes a replicated 32-partition
        # block so the whole cps bank is written (legal step-1 DVE copy after).
        ones = const.tile([128, 32], bf)
        nc.vector.memset(ones[:], 1.0)

        x8 = const.tile([128, 2, ppc * 128], f8)
        nc.sync.dma_start(x8[:], xs8p[:, :, :])
        xg = const.tile([DEXP, ppc * 128], bf)
        nc.sync.dma_start(xg[:], xgp[:, :])
        # rhs slabs: split the big load between the two HWDGE rings, first
        # half (needed first) on its own ring.
        half = (ppc // 2) * 1024
        y8 = const.tile([128, 2, ppc * 1024], f8)
        yg = const.tile([DEXP, ppc * 1024], bf)
        if half:
            nc.scalar.dma_start(y8[:, :, :half], ys8p[:, :, :half])
            nc.sync.dma_start(y8[:, :, half:], ys8p[:, :, half:])
            nc.scalar.dma_start(yg[:, :half], ygp[:, :half])
            nc.sync.dma_start(yg[:, half:], ygp[:, half:])
        else:
            nc.scalar.dma_start(y8[:], ys8p[:, :, :])
            nc.scalar.dma_start(yg[:], ygp[:, :])

        def body():
            acc = accp.tile([128, ppc], f32, tag="acc")
            scs = []
            for p in range(ppc):
                ps = psum.tile([128, 1024], f32, tag="ps", bufs=3)
                for h in range(2):
                    s = 2 * p + h
                    nsl = slice(h * NTILE, (h + 1) * NTILE)
                    nc.tensor.matmul(
                        ps[:, nsl],
                        x8[:, :, p * 128 : (p + 1) * 128],
                        y8[:, :, s * NTILE : (s + 1) * NTILE],
                        start=True, stop=False,
                        perf_mode=mybir.MatmulPerfMode.DoubleRow,
                    )
                    nc.tensor.matmul(
                        ps[:, nsl],
                        xg[:, p * 128 : (p + 1) * 128],
                        yg[:, s * NTILE : (s + 1) * NTILE],
                        start=False, stop=True,
                    )
                # bufs=ppc decouples the exp stage from the col-sum consumers
                # so the per-pair PE->ACT->PE->DVE chain pipelines freely.
                sc = scratch.tile([128, 1024], bf, tag="sc", bufs=ppc)
                nc.scalar.activation(
                    sc[:], ps[:], Exp,
                    scale=1.0 / TEMP,
                    accum_out=acc[:, p : p + 1],
                )
                scs.append(sc)
                if p % 2 == 1 or p == ppc - 1:
                    # col-sums for slots 4g..4g+3: each lands on a 32-partition
                    # block of one psum bank (col-tiling); one step-1 DVE copy
                    # hops PSUM->SBUF, then the DMA gathers rows {0,32,64,96}.
                    g = p // 2
                    lanes = min(4, 2 * (p + 1) - 4 * g)
                    cps = psumc.tile([128, NTILE], f32, tag="cps")
                    for l in range(lanes):
                        s = 4 * g + l
                        psc, hh = scs[s // 2], s % 2
                        nc.tensor.matmul(
                            cps[32 * l : 32 * (l + 1), :],
                            ones[:, :32],
                            psc[:, hh * NTILE : (hh + 1) * NTILE],
                            start=True, stop=True,
                            tile_position=(0, 32 * l),
                        )
                    colsb = scratch.tile([128, NTILE], f32, tag="colsb")
                    nc.vector.tensor_copy(
                        colsb[: 32 * lanes, :], cps[: 32 * lanes, :]
                    )
                    nc.sync.dma_start(
                        uc[g, :lanes, :], colsb[0 : 32 * lanes : 32, :]
                    )
            nc.sync.dma_start(ur[:, :], acc[:])

        if repeat == 1:
            body()
        else:
            with tc.For_i(0, repeat, 1):
                body()

    nc.compile()
    return nc


def _tri_in_maps(npad, yt8, g_onehot, xg_full):
    """Pack per-core operand slabs for the triangle program."""
    pairs = _tri_pairs(npad)
    ppc = len(pairs) // N_CORES
    f8np = yt8.dtype
    in_maps = []
    assign = [pairs[c * ppc : (c + 1) * ppc] for c in range(N_CORES)]
    for c in range(N_CORES):
        xs8p = np.zeros((128, 2, ppc * 128), dtype=f8np)
        xgp = np.zeros((DEXP, ppc * 128), dtype=ml_dtypes.bfloat16)
        ys8p = np.zeros((128, 2, ppc * 1024), dtype=f8np)
        ygp = np.ones((DEXP, ppc * 1024), dtype=ml_dtypes.bfloat16)
        for p, (mi, nis) in enumerate(assign[c]):
            if mi is None:
                xgp[:, p * 128 : (p + 1) * 128] = -PEN
                continue
            xs8p[:, :, p * 128 : (p + 1) * 128] = yt8[:, :, mi * 128 : (mi + 1) * 128]
            xgp[:, p * 128 : (p + 1) * 128] = xg_full[:, mi * 128 : (mi + 1) * 128]
            for h, ni in enumerate(nis):
                s = 2 * p + h
                ys8p[:, :, s * NTILE : (s + 1) * NTILE] = yt8[
                    :, :, ni * NTILE : (ni + 1) * NTILE
                ]
                ygp[:, s * NTILE : (s + 1) * NTILE] = g_onehot[
                    :, ni * NTILE : (ni + 1) * NTILE
                ]
        in_maps.append({"xs8p": xs8p, "xgp": xgp, "ys8p": ys8p, "ygp": ygp})
    return in_maps, assign


def _tri_combine(npad, res, assign):
    """Scatter-add per-core row/col partial sums into U [npad]."""
    u = np.zeros(npad, dtype=np.float64)
    for c in range(N_CORES):
        ur = res[c]["ur"].astype(np.float64)  # [128, ppc]
        ucs = res[c]["uc"].astype(np.float64)  # [n_groups, 4, 512]
        for p, (mi, nis) in enumerate(assign[c]):
            if mi is None:
                continue
            u[mi * 128 : (mi + 1) * 128] += ur[:, p]
            for h, ni in enumerate(nis):
                if ni != mi // 4:  # strictly-upper slot: mirror via col-sums
                    s = 2 * p + h
                    u[ni * NTILE : (ni + 1) * NTILE] += ucs[s // 4, s % 4, :]
    return u


def _build_program(npad: int, repeat: int = 1, fp8: bool = USE_FP8) -> bass.Bass:
    """One SPMD Bass program: each core gets the full column matrix plus its
    own row slice, and writes U partial row-sums [R, 1].

    fp8=True packs the 256-dim emb contraction as fp8e4 DoubleRow (one matmul
    per n-tile instead of two); the graph-penalty K=16 matmul stays bf16.
    repeat > 1 wraps the compute body in a hardware loop (benchmarking only)."""
    rows = npad // N_CORES
    bf = mybir.dt.bfloat16
    f8 = mybir.dt.float8e4
    f32 = mybir.dt.float32
    Exp = mybir.ActivationFunctionType.Exp

    nc = bacc.Bacc(
        "TRN2",
        target_bir_lowering=False,
        debug=False,
        num_devices=N_CORES,
        # keep the BIR free of source-path debug info so the NEFF cache key
        # is independent of where this file lives
        disable_frame_to_traceback=True,
    )
    if fp8:
        yt8 = nc.declare_dram_parameter("yt8", [128, 2, npad], f8, isOutput=False)
        ytg = nc.declare_dram_parameter("ytg", [DEXP, npad], bf, isOutput=False)
        xs8 = nc.declare_dram_parameter("xs8", [128, 2, rows], f8, isOutput=False)
        xsg = nc.declare_dram_parameter("xsg", [DEXP, rows], bf, isOutput=False)
    else:
        yt = nc.declare_dram_parameter("yt", [256 + DEXP, npad], bf, isOutput=False)
        xs = nc.declare_dram_parameter("xs", [256 + DEXP, rows], bf, isOutput=False)
    u = nc.declare_dram_parameter("u", [rows, 1], f32, isOutput=True)

    n_tiles = npad // NTILE
    chunks = [
        (c0 * NTILE, min(CHUNK, n_tiles - c0) * NTILE)
        for c0 in range(0, n_tiles, CHUNK)
    ]
    m_tiles = [(m0, min(128, rows - m0)) for m0 in range(0, rows, 128)]

    with tile.TileContext(nc) as tc, ExitStack() as ctx:
        const = ctx.enter_context(tc.tile_pool(name="const", bufs=1))
        psum = ctx.enter_context(
            tc.tile_pool(name="psum", bufs=2, space=bass.MemorySpace.PSUM)
        )
        scratch = ctx.enter_context(tc.tile_pool(name="scratch", bufs=2))
        accp = ctx.enter_context(tc.tile_pool(name="acc", bufs=2))

        # Warm the exp table while DMAs run.
        dummy_in = const.tile([128, 8], f32)
        nc.vector.memset(dummy_in[:], 0.0)
        dummy_out = const.tile([128, 8], bf)
        nc.scalar.activation(dummy_out[:], dummy_in[:], Exp)

        # Row-slice operands (lhsT). Split loads across the two HWDGE rings
        # (SP=nc.sync, ACT=nc.scalar) so the head isn't serialized on one ring.
        if fp8:
            x8 = const.tile([128, 2, rows], f8)
            nc.sync.dma_start(x8[:], xs8[:, :, :])
            xg = const.tile([DEXP, rows], bf)
            nc.scalar.dma_start(xg[:], xsg[:, :])
            xemb = (x8,)
        else:
            x0 = const.tile([128, rows], bf)
            nc.sync.dma_start(x0[:], xs[0:128, :])
            x1 = const.tile([128, rows], bf)
            nc.scalar.dma_start(x1[:], xs[128:256, :])
            xg = const.tile([DEXP, rows], bf)
            nc.scalar.dma_start(xg[:], xs[256 : 256 + DEXP, :])
            xemb = (x0, x1)

        # Column operands (rhs), one DMA per (k-tile, chunk); alternate rings
        # by chunk so both rings stream concurrently.
        ycols = []
        for ci, (c0, cw) in enumerate(chunks):
            ring = nc.sync if ci % 2 == 0 else nc.scalar
            if fp8:
                y8 = const.tile([128, 2, cw], f8, tag=f"y8_{c0}")
                ring.dma_start(y8[:], yt8[:, :, c0 : c0 + cw])
                yg = const.tile([DEXP, cw], bf, tag=f"yg_{c0}")
                ring.dma_start(yg[:], ytg[:, c0 : c0 + cw])
                ycols.append((y8, yg))
            else:
                y0 = const.tile([128, cw], bf, tag=f"y0_{c0}")
                ring.dma_start(y0[:], yt[0:128, c0 : c0 + cw])
                y1 = const.tile([128, cw], bf, tag=f"y1_{c0}")
                ring.dma_start(y1[:], yt[128:256, c0 : c0 + cw])
                y2 = const.tile([DEXP, cw], bf, tag=f"y2_{c0}")
                ring.dma_start(y2[:], yt[256 : 256 + DEXP, c0 : c0 + cw])
                ycols.append((y0, y1, y2))

        def body():
            _emit_compute(
                nc, tc, m_tiles, chunks, ycols, xemb, xg, u,
                psum, scratch, accp, fp8,
            )

        if repeat == 1:
            body()
        else:
            with tc.For_i(0, repeat, 1):
                body()

    nc.compile()
    return nc


def _emit_compute(nc, tc, m_tiles, chunks, ycols, xemb, xg, u, psum, scratch, accp, fp8):
    bf = mybir.dt.bfloat16
    f32 = mybir.dt.float32
    Exp = mybir.ActivationFunctionType.Exp
    for m0, mw in m_tiles:
        acc = accp.tile([128, len(chunks)], f32)
        for ci, (c0, cw) in enumerate(chunks):
            ps = psum.tile([128, CHUNK * NTILE], f32, tag="ps")
            for t in range(cw // NTILE):
                nsl = slice(t * NTILE, (t + 1) * NTILE)
                if fp8:
                    (x8,) = xemb
                    y8, yg = ycols[ci]
                    nc.tensor.matmul(
                        ps[:mw, nsl],
                        x8[:, :, m0 : m0 + mw],
                        y8[:, :, t * NTILE : (t + 1) * NTILE],
                        start=True, stop=False,
                        perf_mode=mybir.MatmulPerfMode.DoubleRow,
                    )
                else:
                    x0, x1 = xemb
                    y0, y1, yg = ycols[ci]
                    nc.tensor.matmul(
                        ps[:mw, nsl], x0[:, m0 : m0 + mw], y0[:, nsl],
                        start=True, stop=False,
                    )
                    nc.tensor.matmul(
                        ps[:mw, nsl], x1[:, m0 : m0 + mw], y1[:, nsl],
                        start=False, stop=False,
                    )
                nc.tensor.matmul(
                    ps[:mw, nsl], xg[:, m0 : m0 + mw], yg[:, nsl],
                    start=False, stop=True,
                )
            sc = scratch.tile([128, CHUNK * NTILE], bf, tag="sc")
            nc.scalar.activation(
                sc[:mw, :cw], ps[:mw, :cw], Exp,
                scale=1.0 / TEMP,
                accum_out=acc[:mw, ci : ci + 1],
            )
        ured = accp.tile([128, 1], f32, tag="ured")
        nc.vector.tensor_reduce(
            ured[:mw, :], acc[:mw, : len(chunks)],
            axis=mybir.AxisListType.X, op=mybir.AluOpType.add,
        )
        nc.sync.dma_start(u[m0 : m0 + mw, :], ured[:mw, :])


def kernel(embeddings, labels, graph_ids, categories):
    import os
    import time

    _dbg = bool(os.environ.get("KERNEL_DEBUG_TIMING"))
    _t0 = time.time()

    def _mark(msg):
        if _dbg:
            print(f"[kernel] {msg}: {time.time() - _t0:.2f}s", flush=True)

    emb = np.asarray(embeddings, dtype=np.float32)
    lab = np.asarray(labels).astype(np.int64)
    gid = np.asarray(graph_ids).astype(np.int64)
    cat = np.asarray(categories).astype(np.int64)
    n, d = emb.shape
    assert d == 256

    norms = np.linalg.norm(emb, axis=1, keepdims=True)
    e = emb / np.maximum(norms, EPS)

    cons = cat < 3

    # Label groups via sort; a conserved node participates iff its label group
    # has conserved members spanning >=2 distinct graphs.
    order = np.argsort(lab, kind="stable")
    lab_s = lab[order]
    starts = np.flatnonzero(np.r_[True, lab_s[1:] != lab_s[:-1]])
    ends = np.r_[starts[1:], n]

    part_mask = np.zeros(n, dtype=bool)
    cnt = np.zeros(n, dtype=np.int64)  # positive partners per node
    pair_i, pair_j = [], []  # unordered positive pairs
    for s, t in zip(starts, ends):
        idx = order[s:t]
        ci = idx[cons[idx]]
        if len(ci) < 2:
            continue
        gg = gid[ci]
        if (gg == gg[0]).all():
            continue
        part_mask[ci] = True
        # partners: same label, conserved, different graph
        gcounts = {}
        for g in gg:
            gcounts[g] = gcounts.get(g, 0) + 1
        cnt[ci] = len(ci) - np.array([gcounts[g] for g in gg])
        ii, jj = np.triu_indices(len(ci), k=1)
        diff = gg[ii] != gg[jj]
        pair_i.append(ci[ii[diff]])
        pair_j.append(ci[jj[diff]])

    if not pair_i:
        return np.float32(0.0)
    pair_i = np.concatenate(pair_i)
    pair_j = np.concatenate(pair_j)
    n_pairs = len(pair_i)
    if n_pairs == 0:
        return np.float32(0.0)

    _mark("host group prep")
    # Host pair similarities (fp32 like the reference).
    s_pairs = np.einsum("ij,ij->i", e[pair_i], e[pair_j], dtype=np.float64)
    pos_loss = np.sum(1.0 - s_pairs) / n_pairs

    part = np.flatnonzero(part_mask)
    npp = len(part)
    npad = max(NTILE, -(-npp // NTILE) * NTILE)

    # Graph one-hot [16, npad]; padded columns get all-ones so every row
    # sees the -PEN penalty (kills diag, same-graph, and pad columns).
    g_onehot = np.zeros((DEXP, npad), dtype=ml_dtypes.bfloat16)
    g_onehot[gid[part], np.arange(npp)] = 1.0
    g_onehot[:, npp:] = 1.0

    rows = npad // N_CORES
    if USE_TRI:
        f8np = mybir.dt.np(mybir.dt.float8e4)
        e8 = e[part].astype(f8np)
        yt8 = np.zeros((128, 2, npad), dtype=f8np)
        yt8[:, :, :npp] = e8.T.reshape(2, 128, npp).transpose(1, 0, 2)
        xg_full = (g_onehot.astype(np.float32) * -PEN).astype(ml_dtypes.bfloat16)
        in_maps, assign = _tri_in_maps(npad, yt8, g_onehot, xg_full)
        _mark("host arrays built")
        key = (npad, "tri")
        nc = _programs.get(key)
        if nc is None:
            nc = _build_program_tri(npad)
            _programs[key] = nc
        _mark("program built")
        res = run_bass_kernel_spmd(nc, in_maps, core_ids=list(range(N_CORES)))
        _mark("device run done")
        u_full = _tri_combine(npad, res.results, assign)[:npp]
        lse = np.log(np.maximum(u_full, 1e-300))
        n_pos = 2 * n_pairs
        nce = (np.sum(cnt[part] * lse) - 2.0 * np.sum(s_pairs / TEMP)) / n_pos
        return np.float32(pos_loss + nce)
    if USE_FP8:
        f8np = mybir.dt.np(mybir.dt.float8e4)
        e8 = e[part].astype(f8np)  # [npp, 256]
        # DoubleRow packing: [ki, ko, n] = E[n, ki + 128*ko]
        yt8 = np.zeros((128, 2, npad), dtype=f8np)
        yt8[:, :, :npp] = e8.T.reshape(2, 128, npp).transpose(1, 0, 2)
        xg_full = (g_onehot.astype(np.float32) * -PEN).astype(ml_dtypes.bfloat16)
        in_maps = [
            {
                "yt8": yt8,
                "ytg": g_onehot,
                "xs8": np.ascontiguousarray(yt8[:, :, c * rows : (c + 1) * rows]),
                "xsg": np.ascontiguousarray(xg_full[:, c * rows : (c + 1) * rows]),
            }
            for c in range(N_CORES)
        ]
    else:
        ebf = e[part].astype(ml_dtypes.bfloat16)
        yt = np.zeros((256 + DEXP, npad), dtype=ml_dtypes.bfloat16)
        yt[:256, :npp] = ebf.T
        yt[256:, :] = g_onehot
        xt = yt.copy()
        xt[256:, :] = g_onehot.astype(np.float32) * -PEN
        in_maps = [
            {"yt": yt, "xs": np.ascontiguousarray(xt[:, c * rows : (c + 1) * rows])}
            for c in range(N_CORES)
        ]

    _mark("host arrays built")
    key = (npad, USE_FP8)
    nc = _programs.get(key)
    if nc is None:
        nc = _build_program(npad)
        _programs[key] = nc
    _mark("program built")
    res = run_bass_kernel_spmd(nc, in_maps, core_ids=list(range(N_CORES)))
    _mark("device run done")
    u_full = np.concatenate([r["u"].reshape(-1) for r in res.results])[:npp]

    lse = np.log(np.maximum(u_full.astype(np.float64), 1e-300))
    # nce = (sum_i cnt_i * lse_i - sum_ordered_pos logits) / n_pos
    n_pos = 2 * n_pairs
    nce = (np.sum(cnt[part] * lse) - 2.0 * np.sum(s_pairs / TEMP)) / n_pos

    return np.float32(pos_loss + nce)
